# revision 20
# baseline (speedup 1.0000x reference)
"""Trainium2 Bass kernel for 12-head attention (B=4, S=2048, E=768, D=64).

Sharding (8 cores): DP over batch (4) x TP over heads (2 halves of 6).
Core c handles batch b = c>>1 with heads hh*6..hh*6+5, hh = c&1.
Each core computes a partial output projection over its 384 head-dims;
the host unshards by summing the TP pair and adding b_out (the TP
all-reduce), so no on-device collectives are needed.

v3 structure (vs v2): the attention kt-pipeline (scores -> exp -> PV)
is ACT-bound (~1.15us/round vs ~0.86us PE), so all projection work is
drip-fed into the PE slack *during* attention instead of running as a
separate PE-only phase with ACT idle:
- prologue: exp-table-load priming activation, K/Q projections for the
  first head pair (cb0) and V tiles 0-7 only; attention starts ~13us in.
- remaining V tiles, cb1/cb2 K/Q projections and the qc0 output
  projection are emitted as fill pieces, one per attention round, using
  a dedicated 1-bank PSUM fill pool so they never steal the score
  buffers (ops pool shrunk to bufs=1 to free the bank).
- output projection split into [eb, 512-col] pieces (1 bank each), DMA
  out per piece over 3 queues.
"""

import sys

if "/opt/trn_rl_repo" not in sys.path:
    sys.path.insert(0, "/opt/trn_rl_repo")

import numpy as np

import concourse.bass as bass  # noqa: F401
import concourse.mybir as mybir
import concourse.tile as tile
from concourse import bacc
from concourse.bass_utils import run_bass_kernel_spmd

F32 = mybir.dt.float32
BF16 = mybir.dt.bfloat16

B, S, E = 4, 2048, 768
NH, HD = 12, 64
H6 = 6            # heads per core (TP half)
HDIM = H6 * HD    # 384 head-dims per core
KT = S // 128     # 16 k tiles
QC = 1024         # q chunk width
NQC = S // QC     # 2 q chunks
EKT = E // 128    # 6 contraction tiles over the embedding dim


def build_program(loop_n=None):
    nc = bacc.Bacc(
        "TRN2",
        target_bir_lowering=False,
        debug=False,
        enable_asserts=False,
        num_devices=8,
    )
    xbT_d = nc.dram_tensor("xbT", [E, S], BF16, kind="ExternalInput").ap()
    wq_d = nc.dram_tensor("wq", [E, HDIM], BF16, kind="ExternalInput").ap()
    wk_d = nc.dram_tensor("wk", [E, HDIM], BF16, kind="ExternalInput").ap()
    wv_d = nc.dram_tensor("wv", [E, HDIM], BF16, kind="ExternalInput").ap()
    wo_d = nc.dram_tensor("wo", [HDIM, E], BF16, kind="ExternalInput").ap()
    outT_d = nc.dram_tensor("outT", [E, S], BF16, kind="ExternalOutput").ap()

    with tile.TileContext(nc) as tc:
        with (
            tc.tile_pool(name="pw", bufs=1) as pw,
            tc.tile_pool(name="pxT", bufs=1) as pxT,
            tc.tile_pool(name="pqkv", bufs=1) as pqkv,
            tc.tile_pool(name="pep", bufs=14) as pep,
            tc.tile_pool(name="pattn", bufs=2) as pattn,
            tc.tile_pool(name="pmsc", bufs=4) as pmsc,
            tc.tile_pool(name="sps", bufs=2, space="PSUM") as sps,
            tc.tile_pool(name="ops", bufs=1, space="PSUM") as ops,
            tc.tile_pool(name="fps", bufs=2, space="PSUM") as fps,
        ):
            # ---- exp-table priming: first ACT instruction, no deps, so the
            # ~2.7us ACT_TABLE_LOAD overlaps the startup DMAs.
            dmy = pmsc.tile([128, 8], F32, tag="dmy", name="dmy")
            nc.vector.memset(dmy[:], 0.0)
            nc.scalar.activation(
                dmy[:], dmy[:], mybir.ActivationFunctionType.Exp, scale=1.0
            )

            # ---- weight tiles: one SBUF tile + one DMA per weight matrix
            # (each dma_start pays ~630ns on the shared HWDGE queue head, so
            # fewer/bigger transfers win). wX_sb[k] views keep the original
            # [128, HDIM]-per-k-tile addressing.
            wkT = pw.tile([128, EKT * HDIM], BF16, tag="wkT", name="wkT")
            wqT = pw.tile([128, EKT * HDIM], BF16, tag="wqT", name="wqT")
            wvT = pw.tile([128, EKT * HDIM], BF16, tag="wvT", name="wvT")
            woT = pw.tile([128, (HDIM // 128) * E], BF16, tag="woT", name="woT")
            wk_sb = [wkT[:, k * HDIM:(k + 1) * HDIM] for k in range(EKT)]
            wq_sb = [wqT[:, k * HDIM:(k + 1) * HDIM] for k in range(EKT)]
            wv_sb = [wvT[:, k * HDIM:(k + 1) * HDIM] for k in range(EKT)]
            wo_sb = [woT[:, k * E:(k + 1) * E] for k in range(HDIM // 128)]
            xT = [pxT.tile([128, S], BF16, tag=f"xT{k}", name=f"xT{k}") for k in range(EKT)]
            qT = [pqkv.tile([128, S], BF16, tag=f"qT{i}", name=f"qT{i}") for i in range(3)]
            kTt = [pqkv.tile([128, S], BF16, tag=f"kT{i}", name=f"kT{i}") for i in range(3)]
            vt = [pqkv.tile([128, H6 * 65], BF16, tag=f"v{st}", name=f"v{st}")
                  for st in range(KT)]
            # ones columns (col 64 of each 65-block): softmax denominator
            # falls out of the PV matmul
            for st in range(KT):
                dst = vt[st][:, 0:H6 * 65].rearrange("p (h c) -> p h c", c=65)
                nc.vector.memset(dst[:, :, 64:65], 1.0)

            live = {}

            def proj_sub(w_sb, dst, cb, sc, sub, c0=0, c1=512):
                """Third of a projection piece (2 of 6 contraction matmuls);
                the subs share one fill-pool PSUM tile so each emission is
                about one attention round's PE slack. c0:c1 narrows the
                column range (used for the fine-grained first piece)."""
                key = (id(w_sb), cb, sc, c0)
                if sub == 0:
                    live[key] = fps.tile([128, 512], F32, tag="fp", name="pp")
                pk = live[key]
                for k in (2 * sub, 2 * sub + 1):
                    nc.tensor.matmul(
                        pk[:, 0:c1 - c0],
                        w_sb[k][:, cb * 128:(cb + 1) * 128],
                        xT[k][:, sc * 512 + c0:sc * 512 + c1],
                        start=(k == 0), stop=(k == EKT - 1),
                    )
                if sub == 2:
                    nc.vector.tensor_copy(
                        dst[cb][:, sc * 512 + c0:sc * 512 + c1],
                        pk[:, 0:c1 - c0])
                    del live[key]

            def proj_piece(w_sb, dst, cb, sc, c0=0, c1=512):
                for sub in range(3):
                    proj_sub(w_sb, dst, cb, sc, sub, c0, c1)

            def v_tile(st):
                """V projection for key rows [st*128,(st+1)*128)."""
                pv = fps.tile([128, 512], F32, tag="fp", name="pv")
                for k in range(EKT):
                    nc.tensor.matmul(
                        pv[:, 0:HDIM],
                        xT[k][:, st * 128:(st + 1) * 128],
                        wv_sb[k][:],
                        start=(k == 0), stop=(k == EKT - 1),
                    )
                dst = vt[st][:, 0:H6 * 65].rearrange("p (h c) -> p h c", c=65)
                nc.vector.tensor_copy(
                    dst[:, :, 0:64],
                    pv[:, 0:HDIM].rearrange("p (h c) -> p h c", c=64),
                )

            state = {}

            def attn_scores_exp(qc, h, kt):
                """Scores matmuls (row group (h%2)*64) + exp on ACT; returns
                the ee tile for the lagged PV stage."""
                cb, ro = h // 2, (h % 2) * 64
                spp = sps.tile([128, QC], F32, tag="sp", name="spp")
                for hf in range(QC // 512):
                    q0 = qc * QC + hf * 512
                    nc.tensor.matmul(
                        spp[:, hf * 512:(hf + 1) * 512],
                        kTt[cb][ro:ro + 64, kt * 128:(kt + 1) * 128],
                        qT[cb][ro:ro + 64, q0:q0 + 512],
                        start=True, stop=True,
                    )
                ee = pep.tile([128, QC], BF16, tag="e", name="ee")
                nc.scalar.activation(
                    ee[:], spp[:],
                    mybir.ActivationFunctionType.Exp, scale=0.125,
                )
                return ee

            def attn_pv(qc, h, kt, ee):
                """PV accumulate for (qc,h,kt), with the ones-column
                denominator. Allocates the head's accumulator on kt==0
                (waits for the previous head's norm to release the slot)."""
                if kt == 0:
                    state[("op", qc, h)] = ops.tile([65, QC], F32, tag="op",
                                                    name="op")
                op = state[("op", qc, h)]
                for hf in range(QC // 512):
                    nc.tensor.matmul(
                        op[:, hf * 512:(hf + 1) * 512],
                        vt[kt][:, h * 65:h * 65 + 65],
                        ee[:, hf * 512:(hf + 1) * 512],
                        start=(kt == 0), stop=(kt == KT - 1),
                    )

            def attn_norm(qc, h):
                cb, ro = h // 2, (h % 2) * 64
                op = state.pop(("op", qc, h))
                bc = pmsc.tile([64, QC], F32, tag="bc", name="bc")
                nc.vector.reciprocal(bc[0:1, :], op[64:65, :])
                nc.gpsimd.partition_broadcast(bc[:, :], bc[0:1, :])
                nc.vector.tensor_mul(
                    state["attn2"][qc][cb][ro:ro + 64, :], op[0:64, :], bc[:, :])

            def outproj_sub(qc, eb, hf, k3, attn2):
                """One matmul of a [128,512] output-projection piece; copy +
                per-(eb,qc) DMA hang off the last sub."""
                key = ("oq", eb, hf)
                if k3 == 0:
                    live[key] = fps.tile([128, 512], F32, tag="fp", name="oq")
                oq = live[key]
                nc.tensor.matmul(
                    oq[:],
                    wo_sb[k3][:, eb * 128:(eb + 1) * 128],
                    attn2[k3][:, hf * 512:(hf + 1) * 512],
                    start=(k3 == 0), stop=(k3 == HDIM // 128 - 1),
                )
                if k3 < HDIM // 128 - 1:
                    return
                del live[key]
                if hf == 0:
                    state[("osb", eb)] = pmsc.tile(
                        [128, QC], BF16, tag="osb", name="osb")
                osb = state[("osb", eb)]
                nc.vector.tensor_copy(osb[:, hf * 512:(hf + 1) * 512], oq[:])
                if hf == 1:
                    eng = (nc.sync, nc.scalar)[eb % 2]
                    eng.dma_start(
                        outT_d[eb * 128:(eb + 1) * 128, qc * QC:(qc + 1) * QC],
                        osb[:],
                    )

            def outproj_eb_tail(qc, eb, attn2):
                """Tail output projection for one eb row-block: both halves
                into one 2-bank PSUM tile from the (now idle) scores pool,
                one copy, one DMA."""
                oq = sps.tile([128, QC], F32, tag="sp", name="oqt")
                for k3 in range(HDIM // 128):
                    for hf in range(QC // 512):
                        nc.tensor.matmul(
                            oq[:, hf * 512:(hf + 1) * 512],
                            wo_sb[k3][:, eb * 128:(eb + 1) * 128],
                            attn2[k3][:, hf * 512:(hf + 1) * 512],
                            start=(k3 == 0), stop=(k3 == HDIM // 128 - 1),
                        )
                osb = pmsc.tile([128, QC], BF16, tag="osb", name="osb")
                nc.vector.tensor_copy(osb[:], oq[:])
                eng = (nc.sync, nc.scalar)[eb % 2]
                eng.dma_start(
                    outT_d[eb * 128:(eb + 1) * 128, qc * QC:(qc + 1) * QC],
                    osb[:],
                )

            def body():
                # DMAs on the two HWDGE queues only (the gpsimd queue is
                # SWDGE: Pool pays ~1us/descriptor and it wrecked startup).
                # Group weights with the x column-block that is needed at
                # the same time: wk + x(sc0) gate the first K piece.
                dma_q = [0]

                def dma(dst, src):
                    eng = (nc.sync, nc.scalar)[dma_q[0] % 2]
                    dma_q[0] += 1
                    eng.dma_start(dst, src)

                def wdma(dst_tile, src_d, nk, ncols):
                    dma(
                        dst_tile[:].rearrange("p (k c) -> p k c", c=ncols),
                        src_d.rearrange("(k p) c -> p k c", p=128),
                    )

                # order tracks first-use: wk + x(sc0) gate the first K/Q
                # pieces, x(sc1) gates Q01 and early V tiles, wo ~round 96
                wdma(wkT, wk_d, EKT, HDIM)
                for k in range(EKT):
                    dma(xT[k][:, 0:512], xbT_d[k * 128:(k + 1) * 128, 0:512])
                wdma(wqT, wq_d, EKT, HDIM)
                for k in range(EKT):
                    dma(xT[k][:, 512:1024],
                        xbT_d[k * 128:(k + 1) * 128, 512:1024])
                wdma(wvT, wv_d, EKT, HDIM)
                for k in range(EKT):
                    dma(xT[k][:, QC:S], xbT_d[k * 128:(k + 1) * 128, QC:S])
                wdma(woT, wo_d, HDIM // 128, E)

                # prologue: only what the first rounds need. Scores of key
                # tile kt read just K cols kt*128.., so attention starts
                # after a fine-grained K(0,0..128) + Q00/Q01.
                proj_piece(wk_sb, kTt, 0, 0, 0, 128)
                proj_piece(wq_sb, qT, 0, 0)
                proj_piece(wq_sb, qT, 0, 1)
                proj_piece(wk_sb, kTt, 0, 0, 128, 512)

                # Fill work, drained into the PE slack between each round's
                # exp and (lagged) PV matmuls. EMISSION order must respect
                # data deps: scores(h,kt) at slot 96qc+16h+kt reads
                # K(h//2, kt//4) and Q(h//2, qc-cols); pv(kt) emitted at slot
                # +LAG reads V(kt). Hard-deadline pieces (V tiles + rest of
                # K cb0) are pinned to rounds 0-15 (two fills/round early —
                # the inherent front-loaded overload); the rest is a FIFO
                # drained one sub/round from round 16, ordered to meet its
                # (much looser) deadlines.
                LAG = 3
                from collections import defaultdict

                sched = defaultdict(list)
                for st in range(KT):
                    sched[st].append(lambda st=st: v_tile(st))
                for i, (sc, sub) in enumerate(
                        (sc, sub) for sc in (1, 2, 3) for sub in range(3)):
                    sched[i].append(
                        lambda sc=sc, sub=sub: proj_sub(wk_sb, kTt, 0, sc, sub))

                fills = []

                def FP(w_sb, dst, cb, sc):
                    for sub in range(3):
                        fills.append(
                            lambda s=sub: proj_sub(w_sb, dst, cb, sc, s))

                FP(wq_sb, qT, 1, 0)
                FP(wq_sb, qT, 1, 1)
                for sc in range(4):
                    FP(wk_sb, kTt, 1, sc)
                FP(wq_sb, qT, 2, 0)
                FP(wq_sb, qT, 2, 1)
                for sc in range(4):
                    FP(wk_sb, kTt, 2, sc)
                FP(wq_sb, qT, 0, 2)
                FP(wq_sb, qT, 0, 3)
                for cb in (1, 2):
                    FP(wq_sb, qT, cb, 2)
                    FP(wq_sb, qT, cb, 3)

                # flattened round stream with lagged PV: scores/exp of round
                # i outrank the PV backlog behind a head-boundary norm, so
                # ACT stays dense across boundaries.
                R = [(qc, h, kt)
                     for qc in range(NQC) for h in range(H6)
                     for kt in range(KT)]
                state["attn2"] = {}
                ee_ring = {}

                def pv_stage(i):
                    qc, h, kt = R[i]
                    attn_pv(qc, h, kt, ee_ring.pop(i))
                    if kt == KT - 1:
                        attn_norm(qc, h)
                        if h == H6 - 1 and qc == 0:
                            # qc0's output projection becomes fill work
                            # drained during qc1's rounds
                            a = state["attn2"][0]
                            for eb in range(EKT):
                                for hf in range(QC // 512):
                                    for k3 in range(HDIM // 128):
                                        fills.append(
                                            lambda eb=eb, hf=hf, k3=k3:
                                            outproj_sub(0, eb, hf, k3, a))

                def drain(i):
                    th = sched.pop(i, [])
                    if not th and fills:
                        th.append(fills.pop(0))
                    return th

                for i, (qc, h, kt) in enumerate(R):
                    if h == 0 and kt == 0:
                        state["attn2"][qc] = [
                            pattn.tile([128, QC], BF16, tag=f"attn{j}",
                                       name=f"attn{j}_{qc}")
                            for j in range(3)
                        ]
                    ee_ring[i] = attn_scores_exp(qc, h, kt)
                    for fn in drain(i):
                        fn()
                    if i >= LAG:
                        pv_stage(i - LAG)
                for i in range(len(R) - LAG, len(R)):
                    pv_stage(i)
                assert not fills and not sched and not ee_ring and not live

                # qc1 output projection: the tail
                for eb in range(EKT):
                    outproj_eb_tail(1, eb, state["attn2"][1])

            if loop_n is not None:
                with tc.For_i(0, loop_n, 1):
                    body()
            else:
                body()

    nc.compile()
    return nc


class Runner:
    """Compile once, jit once; re-executions reuse the same loaded executable."""

    def __init__(self, nc, n_cores=8):
        import jax
        import numpy as _np
        from jax.sharding import Mesh, PartitionSpec
        from jax.experimental.shard_map import shard_map
        from concourse import bass2jax, mybir as _mb

        bass2jax.install_neuronx_cc_hook()
        self.n_cores = n_cores
        partition_name = nc.partition_id_tensor.name if nc.partition_id_tensor else None
        in_names, out_names, out_avals, zero_shapes = [], [], [], []
        for alloc in nc.m.functions[0].allocations:
            if not isinstance(alloc, _mb.MemoryLocationSet):
                continue
            name = alloc.memorylocations[0].name
            if alloc.kind == "ExternalInput":
                if name != partition_name:
                    in_names.append(name)
            elif alloc.kind == "ExternalOutput":
                shape = tuple(alloc.tensor_shape)
                dtype = _mb.dt.np(alloc.dtype)
                out_avals.append(jax.core.ShapedArray(shape, dtype))
                zero_shapes.append((shape, dtype))
                out_names.append(name)
        self.in_names, self.out_names = list(in_names), list(out_names)
        self.out_avals = out_avals
        self.zero_shapes = zero_shapes
        n_params, n_outs = len(in_names), len(out_names)
        all_names = in_names + out_names
        if partition_name is not None:
            all_names = all_names + [partition_name]

        def _body(*args):
            operands = list(args)
            if partition_name is not None:
                operands.append(bass2jax.partition_id_tensor())
            outs = bass2jax._bass_exec_p.bind(
                *operands,
                out_avals=tuple(out_avals),
                in_names=tuple(all_names),
                out_names=tuple(out_names),
                lowering_input_output_aliases=(),
                sim_require_finite=True,
                sim_require_nnan=True,
                nc=nc,
            )
            return tuple(outs)

        devices = jax.devices()[:n_cores]
        mesh = Mesh(_np.asarray(devices), ("core",))
        in_specs = (PartitionSpec("core"),) * (n_params + n_outs)
        out_specs = (PartitionSpec("core"),) * n_outs
        self._fn = jax.jit(
            shard_map(_body, mesh=mesh, in_specs=in_specs,
                      out_specs=out_specs, check_rep=False),
            donate_argnums=tuple(range(n_params, n_params + n_outs)),
            keep_unused=True,
        )

    def __call__(self, in_maps):
        import numpy as _np
        n = self.n_cores
        concat_in = [
            _np.concatenate([_np.asarray(m[name]) for m in in_maps], axis=0)
            for name in self.in_names
        ]
        concat_zeros = [
            _np.zeros((n * s[0], *s[1:]), d) for (s, d) in self.zero_shapes
        ]
        out_arrs = self._fn(*concat_in, *concat_zeros)
        return [
            {
                name: _np.asarray(out_arrs[i]).reshape(n, *self.out_avals[i].shape)[c]
                for i, name in enumerate(self.out_names)
            }
            for c in range(8)
        ]


_CACHED = {}


def _get_runner(loop_n=None):
    key = loop_n
    if key not in _CACHED:
        _CACHED[key] = Runner(build_program(loop_n))
    return _CACHED[key]


def make_in_maps(x, w_qkv, w_out):
    import ml_dtypes
    bf16 = ml_dtypes.bfloat16
    x = np.ascontiguousarray(x, dtype=np.float32)
    w_qkv = np.ascontiguousarray(w_qkv, dtype=np.float32)
    w_out = np.ascontiguousarray(w_out, dtype=np.float32)
    in_maps = []
    for c in range(8):
        b, hh = c >> 1, c & 1
        cs = hh * HDIM
        in_maps.append({
            "xbT": np.ascontiguousarray(x[b].T.astype(bf16)),
            "wq": np.ascontiguousarray(w_qkv[:, cs:cs + HDIM].astype(bf16)),
            "wk": np.ascontiguousarray(w_qkv[:, E + cs:E + cs + HDIM].astype(bf16)),
            "wv": np.ascontiguousarray(w_qkv[:, 2 * E + cs:2 * E + cs + HDIM].astype(bf16)),
            "wo": np.ascontiguousarray(w_out[cs:cs + HDIM, :].astype(bf16)),
        })
    return in_maps


def run(x, w_qkv, w_out, b_out, loop_n=None):
    in_maps = make_in_maps(x, w_qkv, w_out)
    try:
        results = _get_runner(loop_n)(in_maps)
    except KeyError:
        _CACHED.pop(loop_n, None)
        results = run_bass_kernel_spmd(
            build_program(loop_n), in_maps, list(range(8))
        ).results
    out = np.empty((B, S, E), dtype=np.float32)
    bo = np.asarray(b_out, dtype=np.float32)
    for b in range(B):
        acc = (results[2 * b]["outT"].astype(np.float32)
               + results[2 * b + 1]["outT"].astype(np.float32))
        out[b] = acc.T + bo
    return out


def kernel(x, w_qkv, w_out, b_out):
    return run(x, w_qkv, w_out, b_out)
